# revision 11
# baseline (speedup 1.0000x reference)
"""Causal single-head attention on 8 Trainium2 NeuronCores.

Problem: x[4,4096,1024], Wq/Wk/Wv[1024,64] ->
         softmax(causal(q k^T) / sqrt(64)) @ v  -> [4,4096,64]

Sharding (uniform SPMD program, zero per-core control flow):
  core c = (batch b = c//2, parity v = c%2).
  Flash-decoding-style split of the KEY range: core (b,v) processes
  k-chunks (128 tokens) with global chunk index == v (mod 2), for ALL
  4096 queries of batch b. Partial results (unnormalized numerator O^T
  and softmax denominator row) are combined on the host:
      O = (num_v0 + num_v1) / (den_v0 + den_v1).
  The no-max softmax (exp(s/8) directly, no row-max subtraction) makes
  the partials linearly combinable; scores are ~N(0,1) after scaling so
  exp never overflows fp32.

  Per-core inputs differ only in DATA:
    xt:    x[b]^T with token columns parity-packed (own-parity 128-chunks
           first, then the rest) so K/V projections read a static prefix.
    maskd: the causal mask for the (single, always-last) diagonal chunk
           pair of each q-block, host-computed per parity.
  The program itself is identical on all 8 cores -> one NEFF, one SPMD
  dispatch via run_bass_kernel_spmd.

On-device per core:
  - Projections, fp32r (TF32-class) matmuls at full PE rate:
      [Wk|Wv]-packed pass over the 2048 own-parity tokens -> K^T, V^T
      [Wq] pass over all 4096 tokens -> Q^T   (head dim on partitions)
  - V^T -> V via PE transposes (AV needs k-tokens on partitions); a
    column of ones is appended to V so the AV matmul also produces the
    softmax denominators.
  - Attention per q-block B (512 queries): S^T = K_chunk·Q^T in PSUM,
    exp on the scalar engine (PSUM->SBUF, scale=1/8 fused), causal mask
    multiply on the diagonal pair only, then O^T += V^T·P^T accumulated
    in PSUM over the k-loop.
"""

import os
import numpy as np
from contextlib import ExitStack

import concourse.bass as bass
import concourse.tile as tile
from concourse import mybir, bacc
from concourse.bass_utils import run_bass_kernel_spmd
from concourse.masks import make_identity

B, S, E, D = 4, 4096, 1024, 64
NCORES = 8
QB = 512          # q-block width
NB = S // QB      # 8 q-blocks
NLOC = 16         # local k-chunks per core (parity half of 32)
F32 = mybir.dt.float32
F32R = mybir.dt.float32r
SCALE = float(D) ** -0.5  # 0.125


def host_perm(v: int) -> np.ndarray:
    """Token permutation for core parity v: own-parity 128-chunks first."""
    order = np.arange(S).reshape(S // 128, 128)
    return np.concatenate([order[v::2].ravel(), order[1 - v::2].ravel()])


def host_mask(v: int) -> np.ndarray:
    """[128, 1024] f32 0/1 mask for the last (diagonal) chunk pair of every
    q-block. Element [p, 512*ch + f] guards k = 128*(4B+v+2ch)+p against
    stored query column f of block B (stored col f <-> global q offset
    f ^ (128*v))."""
    p = np.arange(128)[:, None]
    f = np.arange(QB)[None, :]
    fg = f ^ (128 * v)
    mk = np.zeros((128, 2, QB), np.float32)
    for ch in range(2):
        mk[:, ch, :] = (fg >= 128 * v + 256 * ch + p).astype(np.float32)
    return mk.reshape(128, 2 * QB)


def host_unpermute_cols(o: np.ndarray, v: int) -> np.ndarray:
    """Map stored output columns back to global query order."""
    if v == 0:
        return o
    cols = np.arange(S)
    return o[:, cols ^ (128 * v)]


def build_program(repeat: int = 1, variant: str = "full") -> bacc.Bacc:
    """repeat>1 emits the whole computation N times back-to-back (same
    tiles, same output) — used only for time-differencing benchmarks.
    variant: "full" | "dmaonly" | "noattn" | "noav" (ablation benchmarks;
    non-"full" variants produce garbage output)."""
    nc = bacc.Bacc("TRN2", target_bir_lowering=False, debug=False,
                   num_devices=NCORES)
    xt = nc.dram_tensor("xt", [E, S], F32R, kind="ExternalInput").ap()
    wkv = nc.dram_tensor("wkv", [E, 128], F32R, kind="ExternalInput").ap()
    wq = nc.dram_tensor("wq", [E, D], F32R, kind="ExternalInput").ap()
    maskd = nc.dram_tensor("maskd", [128, 2 * QB], F32R,
                           kind="ExternalInput").ap()
    out_t = nc.dram_tensor("out_t", [D + 1, S], F32, kind="ExternalOutput").ap()

    with tile.TileContext(nc) as tc, ExitStack() as ctx:
        sb_w = ctx.enter_context(tc.tile_pool(name="wpool", bufs=1))
        xt_pool = ctx.enter_context(tc.tile_pool(name="xtp", bufs=16))
        p_pool = ctx.enter_context(tc.tile_pool(name="ptp", bufs=6))
        ps_proj = ctx.enter_context(tc.tile_pool(name="psproj", bufs=3,
                                                 space="PSUM"))
        ps_sc = ctx.enter_context(tc.tile_pool(name="pssc", bufs=2,
                                               space="PSUM"))
        ps_out = ctx.enter_context(tc.tile_pool(name="psout", bufs=1,
                                                space="PSUM"))

        wkv_sb = sb_w.tile([128, 8, 128], F32R)
        nc.sync.dma_start(out=wkv_sb, in_=wkv.rearrange("(e p) m -> p e m",
                                                        p=128))
        wq_sb = sb_w.tile([128, 8, D], F32R)
        nc.sync.dma_start(out=wq_sb, in_=wq.rearrange("(e p) m -> p e m",
                                                      p=128))
        mask_sb = sb_w.tile([128, 2 * QB], F32R)
        nc.sync.dma_start(out=mask_sb, in_=maskd)
        ident = sb_w.tile([128, 128], F32)
        make_identity(nc, ident[:])

        kt = sb_w.tile([64, NLOC * 128], F32R)       # K^T, local chunk order
        qt = sb_w.tile([64, S], F32R)                # Q^T, permuted token order
        vt = sb_w.tile([128, NLOC * 128], F32)       # rows 64:128 hold V^T
        v_sb = sb_w.tile([128, NLOC, D + 1], F32R)   # V with ones column
        # ones column (f32r memset fails the ISA check; copy-cast from the
        # preallocated const-1.0 AP instead, which also satisfies the
        # "rounded producer" rule for fp32r matmul inputs)
        nc.vector.tensor_copy(
            v_sb[:, :, D], nc.const_aps.tensor(1.0, [128, NLOC], F32))
        ot_sb = sb_w.tile([D + 1, S], F32)

        # [64, 16 chunks, 2 halves, 128] view of Q^T: block B's queries in
        # global order are chunks (2B, half0), (2B, half1), (2B+1, half0), ...
        qt_r = qt[:].rearrange("p (h c i) -> p c h i", h=2, c=NLOC, i=128)

        # [E, 2, 16, 512] view of xt: chunk e, half h, 512-col block
        xt_r = xt.rearrange("e (h t f) -> e h t f", h=2, t=4, f=QB)

        def proj_stage(i: int):
            """Loads x^T columns for t=i (KV+Q) and t=4+i (Q only) in one
            512KB DMA per emb-chunk (two 2KB spans per partition),
            alternating the two HWDGE rings."""
            t0, t1 = i, 4 + i
            xtiles = []
            for e in range(8):
                xe = xt_pool.tile([128, 2, QB], F32R, name=f"xe_{i}_{e}",
                                  tag="xe")
                eng = nc.sync if e % 2 == 0 else nc.scalar
                eng.dma_start(out=xe[:],
                              in_=xt_r[128 * e:128 * (e + 1), :, i, :])
                xtiles.append(xe)
            if variant == "dmaonly":
                return
            t = t0
            pkv = ps_proj.tile([128, QB], F32, name=f"pkv_{t}", tag="psp")
            for e in range(8):
                nc.tensor.matmul(pkv[:], wkv_sb[:, e, :], xtiles[e][:, 0, :],
                                 start=(e == 0), stop=(e == 7))
            nc.vector.tensor_copy(kt[:, QB * t:QB * (t + 1)], pkv[0:64, :])
            nc.vector.tensor_copy(vt[64:128, QB * t:QB * (t + 1)],
                                  pkv[64:128, :])
            pv = ps_proj.tile([128, QB], F32, name=f"pv_{t}", tag="psp")
            for r in range(4):
                m = 4 * t + r
                nc.tensor.transpose(pv[:, 64 * r:64 * (r + 1)],
                                    vt[64:128, 128 * m:128 * (m + 1)],
                                    ident[64:128, 64:128])
            nc.vector.tensor_copy(v_sb[:, 4 * t:4 * t + 4, 0:D],
                                  pv[:, 0:256])
            for t, hh in ((t0, 0), (t1, 1)):
                pq = ps_proj.tile([128, QB], F32, name=f"pq_{t}", tag="psp")
                for e in range(8):
                    nc.tensor.matmul(pq[0:64, :], wq_sb[:, e, :],
                                     xtiles[e][:, hh, :],
                                     start=(e == 0), stop=(e == 7))
                nc.vector.tensor_copy(qt[:, QB * t:QB * (t + 1)], pq[0:64, :])

        def attn_block(b: int):
            otp = None
            if variant != "noav":
                otp = ps_out.tile([D + 1, QB], F32, name=f"otp_{b}", tag="otp")
            q_ap = qt_r[:, 2 * b:2 * b + 2, :, :]
            for pp in range(b + 1):
                sp = ps_sc.tile([128, 2 * QB], F32, name=f"sp_{b}_{pp}",
                                tag="sp")
                for half in range(2):
                    m = 2 * pp + half
                    nc.tensor.matmul(sp[:, QB * half:QB * (half + 1)],
                                     kt[:, 128 * m:128 * (m + 1)], q_ap,
                                     start=True, stop=True)
                pt = p_pool.tile([128, 2 * QB], F32R, name=f"pt_{b}_{pp}",
                                 tag="pt")
                nc.scalar.activation(pt[:], sp[:],
                                     mybir.ActivationFunctionType.Exp,
                                     scale=SCALE)
                if pp == b:
                    # gpsimd: the DVE is busy with PSUM evacuations and the
                    # GPSIMD engine is otherwise idle
                    nc.gpsimd.tensor_mul(pt[:], pt[:], mask_sb[:])
                if variant == "noav":
                    continue
                for half in range(2):
                    m = 2 * pp + half
                    nc.tensor.matmul(otp[:], v_sb[:, m, :],
                                     pt[:, QB * half:QB * (half + 1)],
                                     start=(pp == 0 and half == 0),
                                     stop=(pp == b and half == 1))
            if variant != "noav":
                nc.vector.tensor_copy(ot_sb[:, QB * b:QB * (b + 1)], otp[:])

        # Interleave projection blocks with attention blocks so attention
        # work becomes available as soon as its K/V/Q inputs exist.
        do_attn = variant in ("full", "noav")
        if variant != "full":
            nc.gpsimd.memset(ot_sb[:], 0.0)  # keep ot_sb written in ablations
        for _rep in range(repeat):
            for i in range(4):
                proj_stage(i)
                if do_attn:
                    attn_block(2 * i)
                    attn_block(2 * i + 1)

        nc.sync.dma_start(out=out_t, in_=ot_sb[:])

    nc.compile()
    return nc


_program_cache = {}


def _get_program() -> bacc.Bacc:
    if "nc" not in _program_cache:
        _program_cache["nc"] = build_program()
    return _program_cache["nc"]


def make_in_maps(x, Wq, Wk, Wv):
    x = np.ascontiguousarray(np.asarray(x, dtype=np.float32))
    wkv_np = np.ascontiguousarray(
        np.concatenate([np.asarray(Wk, np.float32),
                        np.asarray(Wv, np.float32)], axis=1))
    wq_np = np.ascontiguousarray(np.asarray(Wq, np.float32))
    in_maps = []
    for c in range(NCORES):
        b, v = c // 2, c % 2
        xt_c = np.ascontiguousarray(x[b].T[:, host_perm(v)])
        in_maps.append({"xt": xt_c, "wkv": wkv_np, "wq": wq_np,
                        "maskd": host_mask(v)})
    return in_maps


def combine_outputs(per_core_out):
    """per_core_out: list of 8 arrays [65, 4096] (core order)."""
    out = np.empty((B, S, D), np.float32)
    for b in range(B):
        o0 = host_unpermute_cols(np.asarray(per_core_out[2 * b],
                                            np.float32), 0)
        o1 = host_unpermute_cols(np.asarray(per_core_out[2 * b + 1],
                                            np.float32), 1)
        num = o0[:D] + o1[:D]
        den = o0[D] + o1[D]
        out[b] = (num / den).T
    return out


def kernel(x, Wq, Wk, Wv):
    nc = _get_program()
    in_maps = make_in_maps(x, Wq, Wk, Wv)
    res = run_bass_kernel_spmd(nc, in_maps, core_ids=list(range(NCORES)))
    if res.exec_time_ns is not None:
        print(f"HW exec time: {res.exec_time_ns} ns")
    return combine_outputs([r["out_t"] for r in res.results])


# revision 15
# speedup vs baseline: 1.3728x; 1.3728x over previous
"""Causal single-head attention on 8 Trainium2 NeuronCores.

Problem: x[4,4096,1024], Wq/Wk/Wv[1024,64] ->
         softmax(causal(q k^T) / sqrt(64)) @ v  -> [4,4096,64]

Sharding (uniform SPMD program, zero per-core control flow):
  core c = (batch b = c//2, parity v = c%2).
  Flash-decoding-style split of the KEY range: core (b,v) processes
  k-chunks (128 tokens) with global chunk index == v (mod 2), for ALL
  4096 queries of batch b. Partial results (unnormalized numerator O^T
  and softmax denominator row) are combined on the host:
      O = (num_v0 + num_v1) / (den_v0 + den_v1).
  The no-max softmax (exp(s/8) directly, no row-max subtraction) makes
  the partials linearly combinable; scores are ~N(0,1) after scaling so
  exp never overflows fp32.

  Per-core inputs differ only in DATA:
    xt:    x[b]^T with token columns parity-packed (own-parity 128-chunks
           first, then the rest) so K/V projections read a static prefix.
    maskd: the causal mask for the (single, always-last) diagonal chunk
           pair of each q-block, host-computed per parity.
  The program itself is identical on all 8 cores -> one NEFF, one SPMD
  dispatch via run_bass_kernel_spmd.

On-device per core:
  - Projections, fp32r (TF32-class) matmuls at full PE rate:
      [Wk|Wv]-packed pass over the 2048 own-parity tokens -> K^T, V^T
      [Wq] pass over all 4096 tokens -> Q^T   (head dim on partitions)
  - V^T -> V via PE transposes (AV needs k-tokens on partitions); a
    column of ones is appended to V so the AV matmul also produces the
    softmax denominators.
  - Attention per q-block B (512 queries): S^T = K_chunk·Q^T in PSUM,
    exp on the scalar engine (PSUM->SBUF, scale=1/8 fused), causal mask
    multiply on the diagonal pair only, then O^T += V^T·P^T accumulated
    in PSUM over the k-loop.
"""

import os
import numpy as np
from contextlib import ExitStack

import concourse.bass as bass
import concourse.tile as tile
from concourse import mybir, bacc
from concourse.bass_utils import run_bass_kernel_spmd
from concourse.masks import make_identity

B, S, E, D = 4, 4096, 1024, 64
NCORES = 8
QB = 512          # q-block width
NB = S // QB      # 8 q-blocks
NLOC = 16         # local k-chunks per core (parity half of 32)
F32 = mybir.dt.float32
F32R = mybir.dt.float32r
SCALE = float(D) ** -0.5  # 0.125


def host_perm(v: int) -> np.ndarray:
    """Token permutation for core parity v: own-parity 128-chunks first."""
    order = np.arange(S).reshape(S // 128, 128)
    return np.concatenate([order[v::2].ravel(), order[1 - v::2].ravel()])


def host_mask(v: int) -> np.ndarray:
    """[128, 1024] f32 0/1 mask for the last (diagonal) chunk pair of every
    q-block. Element [p, 512*ch + f] guards k = 128*(4B+v+2ch)+p against
    stored query column f of block B (stored col f <-> global q offset
    f ^ (128*v))."""
    p = np.arange(128)[:, None]
    f = np.arange(QB)[None, :]
    fg = f ^ (128 * v)
    mk = np.zeros((128, 2, QB), np.float32)
    for ch in range(2):
        mk[:, ch, :] = (fg >= 128 * v + 256 * ch + p).astype(np.float32)
    return mk.reshape(128, 2 * QB)


def host_unpermute_cols(o: np.ndarray, v: int) -> np.ndarray:
    """Map stored output columns back to global query order."""
    if v == 0:
        return o
    cols = np.arange(S)
    return o[:, cols ^ (128 * v)]


def build_program(repeat: int = 1, variant: str = "full",
                  mask_engine: str = "dve",
                  dma_mode: str = "v1") -> bacc.Bacc:
    """repeat>1 emits the whole computation N times back-to-back (same
    tiles, same output) — used only for time-differencing benchmarks.
    variant: "full" | "dmaonly" | "noattn" | "noav" (ablation benchmarks;
    non-"full" variants produce garbage output)."""
    nc = bacc.Bacc("TRN2", target_bir_lowering=False, debug=False,
                   num_devices=NCORES)
    xt = nc.dram_tensor("xt", [E, S], F32R, kind="ExternalInput").ap()
    wkv = nc.dram_tensor("wkv", [E, 128], F32R, kind="ExternalInput").ap()
    wq = nc.dram_tensor("wq", [E, D], F32R, kind="ExternalInput").ap()
    maskd = nc.dram_tensor("maskd", [128, 2 * QB], F32R,
                           kind="ExternalInput").ap()
    out_t = nc.dram_tensor("out_t", [D + 1, S], F32, kind="ExternalOutput").ap()

    with tile.TileContext(nc) as tc, ExitStack() as ctx:
        sb_w = ctx.enter_context(tc.tile_pool(name="wpool", bufs=1))
        xt_pool = ctx.enter_context(tc.tile_pool(name="xtp", bufs=16))
        p_pool = ctx.enter_context(tc.tile_pool(name="ptp", bufs=6))
        ps_proj = ctx.enter_context(tc.tile_pool(name="psproj", bufs=3,
                                                 space="PSUM"))
        ps_sc = ctx.enter_context(tc.tile_pool(name="pssc", bufs=2,
                                               space="PSUM"))
        ps_out = ctx.enter_context(tc.tile_pool(name="psout", bufs=1,
                                                space="PSUM"))

        wkv_sb = sb_w.tile([128, 8, 128], F32R)
        nc.sync.dma_start(out=wkv_sb, in_=wkv.rearrange("(e p) m -> p e m",
                                                        p=128))
        wq_sb = sb_w.tile([128, 8, D], F32R)
        nc.sync.dma_start(out=wq_sb, in_=wq.rearrange("(e p) m -> p e m",
                                                      p=128))
        mask_sb = sb_w.tile([128, 2 * QB], F32R)
        nc.sync.dma_start(out=mask_sb, in_=maskd)
        ident = sb_w.tile([128, 128], F32)
        make_identity(nc, ident[:])

        kt = sb_w.tile([64, NLOC * 128], F32R)       # K^T, local chunk order
        qt = sb_w.tile([64, S], F32R)                # Q^T, permuted token order
        vt = sb_w.tile([128, NLOC * 128], F32)       # rows 64:128 hold V^T
        v_sb = sb_w.tile([128, NLOC, D + 1], F32R)   # V with ones column
        # ones column (f32r memset fails the ISA check; copy-cast from the
        # preallocated const-1.0 AP instead, which also satisfies the
        # "rounded producer" rule for fp32r matmul inputs)
        nc.vector.tensor_copy(
            v_sb[:, :, D], nc.const_aps.tensor(1.0, [128, NLOC], F32))
        ot_sb = sb_w.tile([D + 1, S], F32)

        # [64, 16 chunks, 2 halves, 128] view of Q^T: block B's queries in
        # global order are chunks (2B, half0), (2B, half1), (2B+1, half0), ...
        qt_r = qt[:].rearrange("p (h c i) -> p c h i", h=2, c=NLOC, i=128)

        # [E, 2, 16, 512] view of xt: chunk e, half h, 512-col block
        xt_r = xt.rearrange("e (h t f) -> e h t f", h=2, t=4, f=QB)

        def proj_stage(i: int):
            """Loads x^T columns for t=i (KV+Q) and t=4+i (Q only) in one
            512KB DMA per emb-chunk (two 2KB spans per partition),
            alternating the two HWDGE rings."""
            t0, t1 = i, 4 + i
            xtiles = []
            for e in range(8):
                xe = xt_pool.tile([128, 2, QB], F32R, name=f"xe_{i}_{e}",
                                  tag="xe")
                if dma_mode == "v2":
                    eng = nc.sync if e % 2 == 0 else nc.scalar
                    eng.dma_start(out=xe[:],
                                  in_=xt_r[128 * e:128 * (e + 1), :, i, :])
                elif dma_mode == "v2sync":
                    nc.sync.dma_start(out=xe[:],
                                      in_=xt_r[128 * e:128 * (e + 1), :, i, :])
                elif dma_mode == "v1dual":
                    for hh, t in ((0, t0), (1, t1)):
                        eng = nc.sync if (e + hh) % 2 == 0 else nc.scalar
                        eng.dma_start(
                            out=xe[:, hh, :],
                            in_=xt[128 * e:128 * (e + 1),
                                   QB * t:QB * (t + 1)])
                else:  # v1: two separate 256KB contiguous DMAs, sync ring
                    for hh, t in ((0, t0), (1, t1)):
                        nc.sync.dma_start(
                            out=xe[:, hh, :],
                            in_=xt[128 * e:128 * (e + 1),
                                   QB * t:QB * (t + 1)])
                xtiles.append(xe)
            if variant == "dmaonly":
                return
            t = t0
            pkv = ps_proj.tile([128, QB], F32, name=f"pkv_{t}", tag="psp")
            for e in range(8):
                nc.tensor.matmul(pkv[:], wkv_sb[:, e, :], xtiles[e][:, 0, :],
                                 start=(e == 0), stop=(e == 7))
            nc.vector.tensor_copy(kt[:, QB * t:QB * (t + 1)], pkv[0:64, :])
            nc.vector.tensor_copy(vt[64:128, QB * t:QB * (t + 1)],
                                  pkv[64:128, :])
            pv = ps_proj.tile([128, QB], F32, name=f"pv_{t}", tag="psp")
            for r in range(4):
                m = 4 * t + r
                nc.tensor.transpose(pv[:, 64 * r:64 * (r + 1)],
                                    vt[64:128, 128 * m:128 * (m + 1)],
                                    ident[64:128, 64:128])
            nc.vector.tensor_copy(v_sb[:, 4 * t:4 * t + 4, 0:D],
                                  pv[:, 0:256])
            for t, hh in ((t0, 0), (t1, 1)):
                pq = ps_proj.tile([128, QB], F32, name=f"pq_{t}", tag="psp")
                for e in range(8):
                    nc.tensor.matmul(pq[0:64, :], wq_sb[:, e, :],
                                     xtiles[e][:, hh, :],
                                     start=(e == 0), stop=(e == 7))
                nc.vector.tensor_copy(qt[:, QB * t:QB * (t + 1)], pq[0:64, :])

        def attn_block(b: int):
            otp = None
            if variant != "noav":
                otp = ps_out.tile([D + 1, QB], F32, name=f"otp_{b}", tag="otp")
            q_ap = qt_r[:, 2 * b:2 * b + 2, :, :]
            for pp in range(b + 1):
                sp = ps_sc.tile([128, 2 * QB], F32, name=f"sp_{b}_{pp}",
                                tag="sp")
                for half in range(2):
                    m = 2 * pp + half
                    nc.tensor.matmul(sp[:, QB * half:QB * (half + 1)],
                                     kt[:, 128 * m:128 * (m + 1)], q_ap,
                                     start=True, stop=True)
                pt = p_pool.tile([128, 2 * QB], F32R, name=f"pt_{b}_{pp}",
                                 tag="pt")
                if variant == "noexp":
                    nc.vector.tensor_copy(pt[:], sp[:])
                else:
                    nc.scalar.activation(pt[:], sp[:],
                                         mybir.ActivationFunctionType.Exp,
                                         scale=SCALE)
                if pp == b and variant != "nomask":
                    eng = nc.gpsimd if mask_engine == "gpsimd" else nc.vector
                    eng.tensor_mul(pt[:], pt[:], mask_sb[:])
                if variant == "noav":
                    continue
                for half in range(2):
                    m = 2 * pp + half
                    nc.tensor.matmul(otp[:], v_sb[:, m, :],
                                     pt[:, QB * half:QB * (half + 1)],
                                     start=(pp == 0 and half == 0),
                                     stop=(pp == b and half == 1))
            if variant != "noav":
                nc.vector.tensor_copy(ot_sb[:, QB * b:QB * (b + 1)], otp[:])

        # Interleave projection blocks with attention blocks so attention
        # work becomes available as soon as its K/V/Q inputs exist.
        do_attn = variant in ("full", "noav", "nomask", "noexp",
                              "serialattn")
        if variant != "full":
            nc.gpsimd.memset(ot_sb[:], 0.0)  # keep ot_sb written in ablations
        for _rep in range(repeat):
            if variant == "serialattn":
                for i in range(4):
                    proj_stage(i)
                for b in range(NB):
                    attn_block(b)
            else:
                for i in range(4):
                    proj_stage(i)
                    if do_attn:
                        attn_block(2 * i)
                        attn_block(2 * i + 1)

        nc.sync.dma_start(out=out_t, in_=ot_sb[:])

    nc.compile()
    return nc


_program_cache = {}


def _get_program() -> bacc.Bacc:
    if "nc" not in _program_cache:
        _program_cache["nc"] = build_program()
    return _program_cache["nc"]


def make_in_maps(x, Wq, Wk, Wv):
    x = np.ascontiguousarray(np.asarray(x, dtype=np.float32))
    wkv_np = np.ascontiguousarray(
        np.concatenate([np.asarray(Wk, np.float32),
                        np.asarray(Wv, np.float32)], axis=1))
    wq_np = np.ascontiguousarray(np.asarray(Wq, np.float32))
    in_maps = []
    for c in range(NCORES):
        b, v = c // 2, c % 2
        xt_c = np.ascontiguousarray(x[b].T[:, host_perm(v)])
        in_maps.append({"xt": xt_c, "wkv": wkv_np, "wq": wq_np,
                        "maskd": host_mask(v)})
    return in_maps


def combine_outputs(per_core_out):
    """per_core_out: list of 8 arrays [65, 4096] (core order)."""
    out = np.empty((B, S, D), np.float32)
    for b in range(B):
        o0 = host_unpermute_cols(np.asarray(per_core_out[2 * b],
                                            np.float32), 0)
        o1 = host_unpermute_cols(np.asarray(per_core_out[2 * b + 1],
                                            np.float32), 1)
        num = o0[:D] + o1[:D]
        den = o0[D] + o1[D]
        out[b] = (num / den).T
    return out


def kernel(x, Wq, Wk, Wv):
    nc = _get_program()
    in_maps = make_in_maps(x, Wq, Wk, Wv)
    res = run_bass_kernel_spmd(nc, in_maps, core_ids=list(range(NCORES)))
    if res.exec_time_ns is not None:
        print(f"HW exec time: {res.exec_time_ns} ns")
    return combine_outputs([r["out_t"] for r in res.results])
